# revision 2
# baseline (speedup 1.0000x reference)
"""MoE (16 experts, top-1 gate, D=H=768) Trainium2 kernel.

Strategy (expert-parallel, per the sharding hint):
  - Host computes the gate (logits argmax) — this IS the dispatch step that
    decides the sharding: tokens are routed to the core owning their expert.
  - 16 experts are sharded 2-per-core across the 8 NeuronCores. Experts are
    sorted by routed-token count: the 8 largest go in slot 0 (capacity C0),
    the 8 smallest in slot 1 (capacity C1 <= C0), so every core does the
    identical padded work and padding waste is minimized. Capacities are
    rounded to 32 columns (the matmul free dim has no 128 constraint).
  - Each core runs the two-GEMM MLP (x @ W1.T -> GELU -> @ W2.T) for its two
    experts over its routed tokens, padded to the slot capacity.
  - Host scatters per-token outputs back to the full [B, N, D] tensor.

Device kernel details:
  - Matmul operands are fp16 (PE full rate + FWL weight loads; fp32
    LDWEIGHTS cannot pipeline and halves matmul throughput; fp16 has 10
    mantissa bits -> rel err ~4e-4 end to end). PSUM accumulation is fp32,
    biases/GELU applied on fp32 PSUM. Outputs are written back fp16 (host
    converts) to halve output HBM traffic.
  - Layouts are pre-transposed on host so the device only does contiguous
    DMAs: the first GEMM computes H^T = W1 @ x^T accumulating over six
    128-row d-chunks; GELU(+b1) is applied PSUM->SBUF on the scalar engine;
    the second GEMM computes Y^T = W2 @ G the same way, then a per-partition
    b2 add on the vector engine (fp32 PSUM -> fp16 SBUF), then DMA out.
  - DMA ring assignment: weights for slot 0 + w1 slot 1 ride the SP HWDGE
    ring; x, biases and w2 slot 1 ride the ACT HWDGE ring; outputs ride the
    GPSIMD SWDGE ring so they never queue behind inputs and their issue cost
    stays off the scalar engine (which runs GELU).
  - All wait-free input DMA instructions are hoisted (BIR rewrite) to the
    head of 'main', ahead of the tile-context entry barrier, so input data
    streams during part of the fixed engine-boot preamble.
  - A few dummy matmuls on a memset tile run while input DMAs stream, so
    the PE HAM clock gate (cold 1.2 GHz -> warm 2.4 GHz after ~3.4 us of
    sustained activity) un-throttles before/while the real matmuls start.
"""

import json

import ml_dtypes
import numpy as np

import concourse.bass as bass
import concourse.mybir as mybir
import concourse.tile as tile
from concourse.bass_utils import run_bass_kernel_spmd

E = 16          # experts
D = 768         # d_model
H = 768         # d_hidden
NCORES = 8
EPC = E // NCORES   # experts (slots) per core = 2
DC = D // 128       # 6 d-chunks
HC = H // 128       # 6 h-chunks

MM_DTYPE = "f16"   # "f16" | "bf16" | "f32r"
N_WARM = 4          # dummy warm-up matmuls (HAM un-throttle)

F32 = mybir.dt.float32
F16 = mybir.dt.float16


def _mm_dt():
    if MM_DTYPE == "f16":
        # fp16 runs at the same PE rate as bf16 (1 col/cycle + FWL weight
        # loads) but has 10 mantissa bits instead of 7 — ~6x lower rounding
        # error. All operands here (|x| < ~6, |W| < ~0.2, GELU outputs) are
        # far inside fp16 range and accumulation is fp32 PSUM.
        return mybir.dt.float16, np.float16
    if MM_DTYPE == "bf16":
        return mybir.dt.bfloat16, ml_dtypes.bfloat16
    return mybir.dt.float32r, np.float32


def _split_multi_waits(bir):
    """Walrus (this image's build) rejects >1 sem-wait on one instruction
    ("Too many sync wait commands" on the TileContext-exit Drain). Move
    excess waits onto a chain of same-engine NoOps directly before the
    instruction — the sequencer runs them in program order, so the
    happens-after relation is preserved exactly."""
    nid = 0
    for fn in bir["functions"]:
        for blk in fn["blocks"]:
            out = []
            for ins in blk["instructions"]:
                si = ins.get("sync_info")
                waits = (si or {}).get("on_wait") or []
                if len(waits) > 1:
                    for w in waits[:-1]:
                        nid += 1
                        out.append({
                            "debug": ins.get("debug", 0),
                            "name": f"I-waitfix{nid}",
                            "opcode": "NoOp",
                            "engine": ins["engine"],
                            "ins": [],
                            "outs": [],
                            "sync_info": {"on_update": [], "on_wait": [w]},
                        })
                    si["on_wait"] = waits[-1:]
                out.append(ins)
            blk["instructions"] = out
    return bir


def _hoist_input_dmas(bir):
    """Move wait-free DMACopy instructions from the tile block to 'main',
    directly before each engine's first Drain (the tile-entry barrier), so
    input data streams during part of the fixed boot preamble. Per-engine
    program order (and hence DMA ring order / semaphore accounting) is
    preserved exactly."""
    for fn in bir["functions"]:
        blocks = {b["name"]: b for b in fn["blocks"]}
        main = blocks.get("main")
        tbs = [b for n, b in blocks.items()
               if n != "main" and not n.endswith("_end")]
        if main is None or len(tbs) != 1:
            continue
        tb = tbs[0]
        hoisted, kept = [], []
        for ins in tb["instructions"]:
            si = ins.get("sync_info") or {}
            if ins["opcode"] == "DMACopy" and not si.get("on_wait"):
                hoisted.append(ins)
            else:
                kept.append(ins)
        if not hoisted:
            continue
        out, done = [], set()
        for ins in main["instructions"]:
            if ins["opcode"] == "Drain" and ins["engine"] not in done:
                done.add(ins["engine"])
                out.extend(h for h in hoisted if h["engine"] == ins["engine"])
            out.append(ins)
        leftover = [h for h in hoisted if h["engine"] not in done]
        if leftover:
            continue  # unexpected engine: leave BIR unmodified for safety
        tb["instructions"] = kept
        main["instructions"] = out
    return bir


def _finalize(nc):
    bir = json.loads(nc.to_json_bytes())
    bir = _split_multi_waits(bir)
    bir = _hoist_input_dmas(bir)
    data = json.dumps(bir).encode()
    nc.to_json_bytes = lambda: data
    return nc


def _chunking(C):
    chunks = []
    c0 = 0
    while c0 < C:
        cw = min(512, C - c0)
        chunks.append((c0, cw))
        c0 += cw
    return chunks


def _build(C0, C1):
    """Per-core SPMD kernel: slot 0 with token capacity C0, slot 1 with C1
    (both multiples of 32). Token dim in chunks of <=512 (PSUM bank limit
    for fp32 accumulation)."""
    caps = [C0, C1]
    slot_chunks = [_chunking(C) for C in caps]

    MMDT, _ = _mm_dt()

    nc = bass.Bass("TRN2", target_bir_lowering=False, debug=False,
                   num_devices=NCORES)
    # Layouts match the SBUF tiles exactly (partition-major) so every DMA is
    # a large contiguous burst.
    xts_d = [nc.dram_tensor(f"xt{s}", [128, DC, caps[s]], MMDT,
                            kind="ExternalInput") for s in range(EPC)]
    yts_d = [nc.dram_tensor(f"yt{s}", [128, DC, caps[s]], F16,
                            kind="ExternalOutput") for s in range(EPC)]
    w1t = nc.dram_tensor("w1t", [EPC, 128, DC, H], MMDT, kind="ExternalInput")
    w2t = nc.dram_tensor("w2t", [EPC, 128, HC, D], MMDT, kind="ExternalInput")
    # biases packed into one [128, EPC*(HC+DC)] f32 tensor: per slot s the
    # columns are [b1 cols (HC), b2 cols (DC)].
    ball = nc.dram_tensor("ball", [128, EPC * (HC + DC)], F32,
                          kind="ExternalInput")

    GELU = mybir.ActivationFunctionType.Gelu

    with tile.TileContext(nc) as tc:
        with (
            tc.tile_pool(name="xp", bufs=1) as xp,
            tc.tile_pool(name="wp", bufs=1) as wp,
            tc.tile_pool(name="gp", bufs=2) as gp,
            tc.tile_pool(name="yp", bufs=3) as yp,
            tc.tile_pool(name="bp", bufs=1) as bp,
            tc.tile_pool(name="pp", bufs=4, space="PSUM") as pp,
        ):
            # ---- HAM warm-up: a memset dummy tile + a few matmuls whose
            # results are never read. They keep the PE busy while input DMAs
            # stream, so the HAM clock gate reaches 8/8 before real matmuls.
            dummy = xp.tile([128, 512], MMDT, tag="warm", name="dummy")
            nc.gpsimd.memset(dummy[:, :], 0.0)
            wps = pp.tile([128, 512], F32, tag="pwarm", bufs=1, name="wps")
            for _ in range(N_WARM):
                nc.tensor.matmul(wps[:, :], dummy[:, 0:128], dummy[:, :],
                                 start=True, stop=True)

            # ---- phase 1: issue ALL input DMAs. No compute-dependent wait
            # ever enters any input ring, so they stream continuously.
            # sync (SP ring): w1 slot0, w2 slot0, w1 slot1  (2 pieces each)
            # scalar (ACT ring): x slot0 (2 pieces), x slot1, biases,
            #                    w2 slot1 (2 pieces)
            tiles = []
            for s in range(EPC):
                w1s = wp.tile([128, DC, H], MMDT, tag=f"w1_{s}",
                              name=f"w1s_{s}")
                w2s = wp.tile([128, HC, D], MMDT, tag=f"w2_{s}",
                              name=f"w2s_{s}")
                xcs = [xp.tile([128, DC, 512], MMDT, tag=f"x_{s}_{ci}",
                               name=f"xc_{s}_{ci}")
                       for ci in range(len(slot_chunks[s]))]
                tiles.append((w1s, w2s, xcs))
            bt = bp.tile([128, EPC * (HC + DC)], F32, tag="b", name="bt")

            (w1s0, w2s0, xcs0), (w1s1, w2s1, xcs1) = tiles
            # SP ring: ordered so slot0's first-GEMM closure lands first.
            nc.sync.dma_start(w1s0[:, 0:2], w1t.ap()[0, :, 0:2])
            nc.sync.dma_start(w1s0[:, 2:6], w1t.ap()[0, :, 2:6])
            nc.sync.dma_start(w2s0[:, 0:3], w2t.ap()[0, :, 0:3])
            nc.sync.dma_start(w2s0[:, 3:6], w2t.ap()[0, :, 3:6])
            nc.sync.dma_start(w1s1[:, 0:3], w1t.ap()[1, :, 0:3])
            nc.sync.dma_start(w1s1[:, 3:6], w1t.ap()[1, :, 3:6])
            # ACT ring: x first (needed first), then biases, then w2 slot1.
            for ci, (c0, cw) in enumerate(slot_chunks[0]):
                nc.scalar.dma_start(xcs0[ci][:, 0:2, :cw],
                                    xts_d[0].ap()[:, 0:2, c0:c0 + cw])
                nc.scalar.dma_start(xcs0[ci][:, 2:6, :cw],
                                    xts_d[0].ap()[:, 2:6, c0:c0 + cw])
            for ci, (c0, cw) in enumerate(slot_chunks[1]):
                nc.scalar.dma_start(xcs1[ci][:, :, :cw],
                                    xts_d[1].ap()[:, :, c0:c0 + cw])
            nc.scalar.dma_start(bt[:, :], ball.ap())
            nc.scalar.dma_start(w2s1[:, 0:3], w2t.ap()[1, :, 0:3])
            nc.scalar.dma_start(w2s1[:, 3:6], w2t.ap()[1, :, 3:6])

            # ---- phase 2: compute
            for s in range(EPC):
                chunks = slot_chunks[s]
                w1s, w2s, xcs = tiles[s]
                b1col = lambda hc: bt[:, s * (HC + DC) + hc:
                                      s * (HC + DC) + hc + 1]
                b2col = lambda dc: bt[:, s * (HC + DC) + HC + dc:
                                      s * (HC + DC) + HC + dc + 1]
                last_slot = (s == EPC - 1)
                for ci, (c0, cw) in enumerate(chunks):
                    xc = xcs[ci]
                    last_chunk = last_slot and (ci == len(chunks) - 1)
                    gc = gp.tile([128, HC, 512], MMDT, tag="g")
                    for hc in range(HC):
                        ps = pp.tile([128, 512], F32, tag="ps")
                        for dc in range(DC):
                            nc.tensor.matmul(
                                ps[:, :cw],
                                w1s[:, dc, hc * 128:(hc + 1) * 128],
                                xc[:, dc, :cw],
                                start=(dc == 0), stop=(dc == DC - 1),
                            )
                        nc.scalar.activation(gc[:, hc, :cw], ps[:, :cw], GELU,
                                             bias=b1col(hc), scale=1.0)
                    # second GEMM; outputs grouped 3 d-chunks per DMA for
                    # bandwidth, except the very last group which flushes
                    # per-d-chunk so the tail pipeline drains early.
                    for g2 in range(2):
                        dl, dh = 3 * g2, 3 * (g2 + 1)
                        split_out = last_chunk and g2 == 1
                        yc = yp.tile([128, 3, 512], F16, tag="y",
                                     name=f"yc_{s}_{ci}_{g2}")
                        for dc in range(dl, dh):
                            ps2 = pp.tile([128, 512], F32, tag="ps")
                            for hc in range(HC):
                                nc.tensor.matmul(
                                    ps2[:, :cw],
                                    w2s[:, hc, dc * 128:(dc + 1) * 128],
                                    gc[:, hc, :cw],
                                    start=(hc == 0), stop=(hc == HC - 1),
                                )
                            nc.vector.tensor_scalar_add(
                                yc[:, dc - dl, :cw], ps2[:, :cw], b2col(dc))
                            if split_out:
                                nc.gpsimd.dma_start(
                                    yts_d[s].ap()[:, dc, c0:c0 + cw],
                                    yc[:, dc - dl, :cw])
                        if not split_out:
                            nc.gpsimd.dma_start(
                                yts_d[s].ap()[:, dl:dh, c0:c0 + cw],
                                yc[:, :, :cw])

    return _finalize(nc)


_NC_CACHE = {}


def _get_nc(C0, C1):
    key = (C0, C1, MM_DTYPE)
    nc = _NC_CACHE.get(key)
    if nc is None:
        nc = _build(C0, C1)
        _NC_CACHE[key] = nc
    return nc


def _cap(n):
    return int(max(64, -(-int(n) // 32) * 32))


def kernel(x, W1, b1, W2, b2, Wg, bg):
    x = np.ascontiguousarray(np.asarray(x, dtype=np.float32))
    W1 = np.asarray(W1, dtype=np.float32)
    b1 = np.asarray(b1, dtype=np.float32)
    W2 = np.asarray(W2, dtype=np.float32)
    b2 = np.asarray(b2, dtype=np.float32)
    Wg = np.asarray(Wg, dtype=np.float32)
    bg = np.asarray(bg, dtype=np.float32)

    B, N, Dx = x.shape
    assert Dx == D and W1.shape == (E, H, D)
    T = B * N
    t = x.reshape(T, D)

    # --- gate / dispatch (host): this decides the sharding ---
    logits = t @ Wg.T + bg
    idx = np.argmax(logits, axis=1)

    counts = np.bincount(idx, minlength=E)
    # slot 0 <- 8 largest experts, slot 1 <- 8 smallest
    order = np.argsort(-counts, kind="stable")
    slot_experts = [order[:NCORES], order[NCORES:]]
    C0 = _cap(counts[slot_experts[0]].max())
    C1 = _cap(counts[slot_experts[1]].max())
    caps = [C0, C1]
    nc = _get_nc(C0, C1)
    _, npdt = _mm_dt()

    tok_ids = [np.nonzero(idx == e)[0] for e in range(E)]

    # --- host-side layout prep ---
    t_mm = t.astype(npdt)
    # w1t[e, i, dc, h] = W1[e, h, dc*128+i] (partition-major, chunk, col)
    w1t_all = np.ascontiguousarray(
        W1.astype(npdt).transpose(0, 2, 1).reshape(E, DC, 128, H)
        .transpose(0, 2, 1, 3))
    w2t_all = np.ascontiguousarray(
        W2.astype(npdt).transpose(0, 2, 1).reshape(E, HC, 128, D)
        .transpose(0, 2, 1, 3))
    # b1c[e, i, hc] = b1[e, hc*128+i]
    b1c_all = np.ascontiguousarray(b1.reshape(E, HC, 128).transpose(0, 2, 1))
    b2c_all = np.ascontiguousarray(b2.reshape(E, DC, 128).transpose(0, 2, 1))

    in_maps = []
    for c in range(NCORES):
        experts = [int(slot_experts[s][c]) for s in range(EPC)]
        ball = np.empty((128, EPC * (HC + DC)), np.float32)
        for s in range(EPC):
            ball[:, s * (HC + DC): s * (HC + DC) + HC] = b1c_all[experts[s]]
            ball[:, s * (HC + DC) + HC: (s + 1) * (HC + DC)] = \
                b2c_all[experts[s]]
        m = {
            "w1t": np.ascontiguousarray(w1t_all[experts]),
            "w2t": np.ascontiguousarray(w2t_all[experts]),
            "ball": ball,
        }
        for s in range(EPC):
            C = caps[s]
            xts = np.zeros((128, DC, C), npdt)
            ids = tok_ids[experts[s]]
            n = len(ids)
            if n:
                xts[:, :, :n] = (
                    t_mm[ids].T.reshape(DC, 128, n).transpose(1, 0, 2))
            m[f"xt{s}"] = xts
        in_maps.append(m)

    res = run_bass_kernel_spmd(nc, in_maps, core_ids=list(range(NCORES)))

    out = np.empty((T, D), np.float32)
    for c in range(NCORES):
        for s in range(EPC):
            e = int(slot_experts[s][c])
            ids = tok_ids[e]
            n = len(ids)
            if n:
                yt = res.results[c][f"yt{s}"].astype(np.float32)
                out[ids] = yt.transpose(1, 0, 2).reshape(D, caps[s])[:, :n].T
    return out.reshape(B, N, D)
